# revision 3
# baseline (speedup 1.0000x reference)
"""AFT-Full attention kernel for 8 TRN2 NeuronCores.

Reference computation (S=2048, B=16, D=512):
    q = query @ Wq.T + bq
    k = key @ Wk.T + bk
    v = k @ Wv.T + bv
    num = exp_pb @ (exp(k) * v);  den = exp_pb @ exp(k)   (per batch)
    out = (sigmoid(q) * num / den).transpose(1,0,2) @ Wo.T + bo

Sharding: data-parallel over batch B: 2 batches per core, no collectives.
Math notes:
  - the max-subtractions in the reference cancel exactly in num/den; values are
    small enough that fp32 exp is safe without them.
  - v = k @ Wv.T = key @ (Wv @ Wk).T  -> v computed directly from key with a
    host-folded weight, so k and v share the same stationary operand.
  - bq/bk are absorbed into query/key on the host via inv(W.T); bo is added on
    the host after gather. (All biases are zero for this problem's inputs.)
Matmuls run as float32r (reduced-precision fp32, 1 cycle/row on TensorE).
"""
import sys

sys.path.insert(0, "/opt/trn_rl_repo")

import numpy as np

S, B, D = 2048, 16, 512
NCORES = 8
BLOC = B // NCORES          # 2 batches per core
ST = S // 128               # 16 seq (j) tiles
DT = D // 128               # 4 feature tiles
NI = S // 128               # 16 output (i) tiles

_cache = {}


def _build(use_kv: bool):
    import concourse.bacc as bacc
    import concourse.mybir as mybir
    import concourse.tile as tile
    from concourse.masks import make_identity

    f32 = mybir.dt.float32
    f32r = mybir.dt.float32r
    ACT = mybir.ActivationFunctionType

    nc = bacc.Bacc()

    qT = nc.declare_dram_parameter("qT", [BLOC, D, S], f32r, isOutput=False)
    kT = nc.declare_dram_parameter("kT", [BLOC, D, S], f32r, isOutput=False)
    kTv = (
        nc.declare_dram_parameter("kTv", [BLOC, D, S], f32r, isOutput=False)
        if use_kv
        else kT
    )
    pbT = nc.declare_dram_parameter("pbT", [S, S], f32r, isOutput=False)
    wk = nc.declare_dram_parameter("wk", [D, D], f32r, isOutput=False)
    wvk = nc.declare_dram_parameter("wvk", [D, D], f32r, isOutput=False)
    wq = nc.declare_dram_parameter("wq", [D, D], f32r, isOutput=False)
    wo = nc.declare_dram_parameter("wo", [D, D], f32r, isOutput=False)
    out = nc.declare_dram_parameter("out", [BLOC, S, D], f32, isOutput=True)

    # [din, dout] -> [p, kt, dout] with din = kt*128 + p
    wk_r = wk.rearrange("(kt p) n -> p kt n", p=128)
    wvk_r = wvk.rearrange("(kt p) n -> p kt n", p=128)
    wq_r = wq.rearrange("(kt p) n -> p kt n", p=128)
    wo_r = wo.rearrange("(kt p) n -> p kt n", p=128)

    with tile.TileContext(nc) as tc:
        with (
            tc.tile_pool(name="big", bufs=1) as big,
            tc.tile_pool(name="psum", bufs=1, space="PSUM") as psum,
        ):
            # persistent exp(k) and exp(k)*v, per local batch: [j, c] layout
            # stored as [p, jt, d] with j = jt*128 + p
            E = [big.tile([128, ST, D], f32r, name=f"E{b}") for b in range(BLOC)]
            Ev = [big.tile([128, ST, D], f32r, name=f"Ev{b}") for b in range(BLOC)]
            ident = big.tile([128, 128], f32, name="ident")
            make_identity(nc, ident)

            # ---------------- phase 1: projections k, v -> E, Ev ------------
            with (
                tc.tile_pool(name="ph1", bufs=1) as ph1,
                tc.tile_pool(name="ph1s", bufs=3) as ph1s,
            ):
                wk_sb = ph1.tile([128, DT, D], f32r)
                nc.sync.dma_start(wk_sb[:, :, :], wk_r)
                wvk_sb = ph1.tile([128, DT, D], f32r)
                nc.sync.dma_start(wvk_sb[:, :, :], wvk_r)

                for b in range(BLOC):
                    kT_r = kT[b].rearrange("(kt p) s -> p kt s", p=128)
                    kTv_r = kTv[b].rearrange("(kt p) s -> p kt s", p=128)
                    for jt in range(ST):
                        sl = slice(jt * 128, (jt + 1) * 128)
                        k_sb = ph1s.tile([128, DT, 128], f32r, tag="k_sb")
                        nc.sync.dma_start(k_sb[:, :, :], kT_r[:, :, sl])
                        ps_k = psum.tile([128, D], f32, tag="nd0")
                        for kt in range(DT):
                            nc.tensor.matmul(
                                ps_k[:, :],
                                k_sb[:, kt, :],
                                wk_sb[:, kt, :],
                                start=(kt == 0),
                                stop=(kt == DT - 1),
                            )
                        if use_kv:
                            kv_sb = ph1s.tile([128, DT, 128], f32r, tag="kv_sb")
                            nc.sync.dma_start(kv_sb[:, :, :], kTv_r[:, :, sl])
                        else:
                            kv_sb = k_sb
                        ps_v = psum.tile([128, D], f32, tag="nd2")
                        for kt in range(DT):
                            nc.tensor.matmul(
                                ps_v[:, :],
                                kv_sb[:, kt, :],
                                wvk_sb[:, kt, :],
                                start=(kt == 0),
                                stop=(kt == DT - 1),
                            )
                        # E = exp(k); Ev = E * v
                        nc.scalar.activation(E[b][:, jt, :], ps_k[:, :], ACT.Exp)
                        nc.vector.tensor_mul(
                            Ev[b][:, jt, :], E[b][:, jt, :], ps_v[:, :]
                        )

            # ---------------- phase 2: einsum + gating + output -------------
            with (
                tc.tile_pool(name="ph2", bufs=1) as ph2,
                tc.tile_pool(name="ph2pb", bufs=4) as ph2pb,
                tc.tile_pool(name="ph2s", bufs=3) as ph2s,
                tc.tile_pool(name="fin", bufs=2) as fin,
            ):
                wq_sb = ph2.tile([128, DT, D], f32r)
                nc.sync.dma_start(wq_sb[:, :, :], wq_r)
                wo_sb = ph2.tile([128, DT, D], f32r)
                nc.sync.dma_start(wo_sb[:, :, :], wo_r)

                for it in range(NI):
                    isl = slice(it * 128, (it + 1) * 128)
                    # --- einsum: num/den for both batches, accumulate over j
                    ps_nd = [
                        psum.tile([128, D], f32, tag=f"nd{x}", name=f"nd{x}_{it}")
                        for x in range(2 * BLOC)
                    ]
                    for jt in range(ST):
                        pb_sb = ph2pb.tile([128, 128], f32r, tag="pb_sb")
                        nc.sync.dma_start(
                            pb_sb[:, :], pbT[jt * 128 : (jt + 1) * 128, isl]
                        )
                        for b in range(BLOC):
                            nc.tensor.matmul(
                                ps_nd[2 * b][:, :],
                                pb_sb[:, :],
                                Ev[b][:, jt, :],
                                start=(jt == 0),
                                stop=(jt == ST - 1),
                            )
                            nc.tensor.matmul(
                                ps_nd[2 * b + 1][:, :],
                                pb_sb[:, :],
                                E[b][:, jt, :],
                                start=(jt == 0),
                                stop=(jt == ST - 1),
                            )
                    # --- q projection, gating, output per batch
                    for b in range(BLOC):
                        qT_r = qT[b].rearrange("(kt p) s -> p kt s", p=128)
                        q_sb = ph2s.tile([128, DT, 128], f32r, tag="q_sb")
                        nc.sync.dma_start(q_sb[:, :, :], qT_r[:, :, isl])
                        ps_q = psum.tile([128, D], f32, tag="ps_q")
                        for kt in range(DT):
                            nc.tensor.matmul(
                                ps_q[:, :],
                                q_sb[:, kt, :],
                                wq_sb[:, kt, :],
                                start=(kt == 0),
                                stop=(kt == DT - 1),
                            )
                        sig = fin.tile([128, D], f32, tag="sig")
                        nc.scalar.activation(sig[:, :], ps_q[:, :], ACT.Sigmoid)
                        rec = fin.tile([128, D], f32, tag="rec")
                        nc.vector.reciprocal(rec[:, :], ps_nd[2 * b + 1][:, :])
                        w = fin.tile([128, D], f32, tag="w")
                        nc.vector.tensor_mul(w[:, :], ps_nd[2 * b][:, :], rec[:, :])
                        y = fin.tile([128, D], f32, tag="y")
                        nc.vector.tensor_mul(y[:, :], w[:, :], sig[:, :])
                        # transpose y -> yT [dk, s] (4 blocks into one PSUM bank)
                        ps_t = psum.tile([128, D], f32, tag="ps_t")
                        for kt in range(DT):
                            nc.tensor.transpose(
                                ps_t[:, kt * 128 : (kt + 1) * 128],
                                y[:, kt * 128 : (kt + 1) * 128],
                                ident,
                            )
                        yT = fin.tile([128, DT, 128], f32r, tag="yT")
                        nc.vector.tensor_copy(
                            yT[:, :, :],
                            ps_t[:, :].rearrange("p (kt s) -> p kt s", kt=DT),
                        )
                        # final projection: out[s, dout]
                        ps_o = psum.tile([128, D], f32, tag="ps_o")
                        for kt in range(DT):
                            nc.tensor.matmul(
                                ps_o[:, :],
                                yT[:, kt, :],
                                wo_sb[:, kt, :],
                                start=(kt == 0),
                                stop=(kt == DT - 1),
                            )
                        o_sb = fin.tile([128, D], f32, tag="o_sb")
                        nc.scalar.copy(o_sb[:, :], ps_o[:, :])
                        nc.sync.dma_start(out[b, isl, :], o_sb[:, :])

    nc.compile()
    return nc


def _prep(query, key, Wq, bq, Wk, bk, Wv, bv, pos_bias, Wo, bo):
    """Host-side preprocessing: transposes + bias absorption."""
    query = np.asarray(query, dtype=np.float32)
    key = np.asarray(key, dtype=np.float32)
    Wq = np.asarray(Wq, dtype=np.float32)
    Wk = np.asarray(Wk, dtype=np.float32)
    Wv = np.asarray(Wv, dtype=np.float32)
    Wo = np.asarray(Wo, dtype=np.float32)
    bq = np.asarray(bq, dtype=np.float32)
    bk = np.asarray(bk, dtype=np.float32)
    bv = np.asarray(bv, dtype=np.float32)
    bo = np.asarray(bo, dtype=np.float32)

    Wvk = Wv @ Wk

    # absorb bq/bk into the activations (exact): x' = x + b @ inv(W.T)
    if np.any(bq):
        query = query + np.linalg.solve(Wq, bq).astype(np.float32)
    if np.any(bk):
        key_k = key + np.linalg.solve(Wk, bk).astype(np.float32)
    else:
        key_k = key
    use_kv = bool(np.any(bv)) or bool(np.any(bk))
    if use_kv:
        # v = key @ Wvk.T + (Wv@bk + bv)  -> absorb into a separate key copy
        bv_eff = Wv @ bk + bv
        key_v = key + np.linalg.solve(Wvk, bv_eff).astype(np.float32)
    else:
        key_v = None

    # [S, B, D] -> [B, D, S]
    qT = np.ascontiguousarray(query.transpose(1, 2, 0))
    kT = np.ascontiguousarray(key_k.transpose(1, 2, 0))
    kTv = np.ascontiguousarray(key_v.transpose(1, 2, 0)) if use_kv else None
    # device consumes exp(pos_bias) transposed; the reference's max-subtraction
    # cancels in num/den so plain exp is exact (values are ~N(0, 0.02))
    pbT = np.ascontiguousarray(np.exp(np.asarray(pos_bias, dtype=np.float32)).T)
    wk = np.ascontiguousarray(Wk.T)
    wvk = np.ascontiguousarray(Wvk.T)
    wq = np.ascontiguousarray(Wq.T)
    wo = np.ascontiguousarray(Wo.T)
    return qT, kT, kTv, pbT, wk, wvk, wq, wo, bo, use_kv


def kernel(query, key, Wq, bq, Wk, bk, Wv, bv, pos_bias, Wo, bo):
    from concourse.bass_utils import run_bass_kernel_spmd

    qT, kT, kTv, pbT, wk, wvk, wq, wo, bo, use_kv = _prep(
        query, key, Wq, bq, Wk, bk, Wv, bv, pos_bias, Wo, bo
    )

    if ("nc", use_kv) not in _cache:
        _cache[("nc", use_kv)] = _build(use_kv)
    nc = _cache[("nc", use_kv)]

    in_maps = []
    for c in range(NCORES):
        bsl = slice(c * BLOC, (c + 1) * BLOC)
        m = {
            "qT": qT[bsl],
            "kT": kT[bsl],
            "pbT": pbT,
            "wk": wk,
            "wvk": wvk,
            "wq": wq,
            "wo": wo,
        }
        if use_kv:
            m["kTv"] = kTv[bsl]
        in_maps.append(m)

    res = run_bass_kernel_spmd(nc, in_maps, core_ids=list(range(NCORES)))
    out = np.concatenate([res.results[c]["out"] for c in range(NCORES)], axis=0)
    if np.any(bo):
        out = out + bo
    return out


# revision 4
# speedup vs baseline: 1.6779x; 1.6779x over previous
"""AFT-Full attention kernel for 8 TRN2 NeuronCores.

Reference computation (S=2048, B=16, D=512):
    q = query @ Wq.T + bq
    k = key @ Wk.T + bk
    v = k @ Wv.T + bv
    num = exp_pb @ (exp(k) * v);  den = exp_pb @ exp(k)   (per batch)
    out = (sigmoid(q) * num / den).transpose(1,0,2) @ Wo.T + bo

Sharding: data-parallel over batch B: 2 batches per core, no collectives.
Math notes:
  - the max-subtractions in the reference cancel exactly in num/den; values are
    small enough that fp32 exp is safe without them.
  - v = k @ Wv.T = key @ (Wv @ Wk).T  -> v computed directly from key with a
    host-folded weight, so k and v share the same stationary operand.
  - bq/bk are absorbed into query/key on the host via inv(W.T); bo is added on
    the host after gather. (All biases are zero for this problem's inputs.)
  - exp(pos_bias) is precomputed on the host (it is batch-independent).
Matmuls run in bf16 (1 cycle/row on TensorE; fp32 PSUM accumulation).
The finalize (gating + output projection) of i-tile T is emitted after the
einsum of i-tile T+1 so the TensorEngine never idles waiting for the
vector-engine epilogue.
"""
import sys

sys.path.insert(0, "/opt/trn_rl_repo")

import numpy as np

S, B, D = 2048, 16, 512
NCORES = 8
BLOC = B // NCORES          # 2 batches per core
ST = S // 128               # 16 seq (j) tiles
DT = D // 128               # 4 feature tiles
NI = S // 128               # 16 output (i) tiles

_cache = {}


def _build(use_kv: bool):
    import concourse.bacc as bacc
    import concourse.mybir as mybir
    import concourse.tile as tile
    from concourse.masks import make_identity

    f32 = mybir.dt.float32
    bf16 = mybir.dt.bfloat16
    ACT = mybir.ActivationFunctionType

    nc = bacc.Bacc()

    # activations pre-tiled on host: [b, st, p, kt, 128] with din = kt*128+p
    qT = nc.declare_dram_parameter("qT", [BLOC, ST, 128, DT, 128], bf16, isOutput=False)
    kT = nc.declare_dram_parameter("kT", [BLOC, ST, 128, DT, 128], bf16, isOutput=False)
    kTv = (
        nc.declare_dram_parameter("kTv", [BLOC, ST, 128, DT, 128], bf16, isOutput=False)
        if use_kv
        else kT
    )
    # exp(pos_bias).T pre-tiled: [it, p, jt, 128i] = expPbT[jt*128+p, it*128+i]
    pbt = nc.declare_dram_parameter("pbt", [NI, 128, ST, 128], bf16, isOutput=False)
    wk = nc.declare_dram_parameter("wk", [D, D], bf16, isOutput=False)
    wvk = nc.declare_dram_parameter("wvk", [D, D], bf16, isOutput=False)
    wq = nc.declare_dram_parameter("wq", [D, D], bf16, isOutput=False)
    wo = nc.declare_dram_parameter("wo", [D, D], bf16, isOutput=False)
    out = nc.declare_dram_parameter("out", [BLOC, S, D], f32, isOutput=True)

    wk_r = wk.rearrange("(kt p) n -> p kt n", p=128)
    wvk_r = wvk.rearrange("(kt p) n -> p kt n", p=128)
    wq_r = wq.rearrange("(kt p) n -> p kt n", p=128)
    wo_r = wo.rearrange("(kt p) n -> p kt n", p=128)

    with tile.TileContext(nc) as tc:
        with (
            tc.tile_pool(name="big", bufs=1) as big,
            tc.tile_pool(name="psum", bufs=1, space="PSUM") as psum,
        ):
            # persistent exp(k), exp(k)*v per local batch: [p, jt, d], j = jt*128+p
            E = [big.tile([128, ST, D], bf16, name=f"E{b}") for b in range(BLOC)]
            Ev = [big.tile([128, ST, D], bf16, name=f"Ev{b}") for b in range(BLOC)]
            ident = big.tile([128, 128], bf16, name="ident")
            make_identity(nc, ident)

            # ---------------- phase 1: projections k, v -> E, Ev ------------
            with (
                tc.tile_pool(name="ph1", bufs=1) as ph1,
                tc.tile_pool(name="ph1s", bufs=3) as ph1s,
            ):
                wk_sb = ph1.tile([128, DT, D], bf16)
                nc.sync.dma_start(wk_sb[:, :, :], wk_r)
                wvk_sb = ph1.tile([128, DT, D], bf16)
                nc.sync.dma_start(wvk_sb[:, :, :], wvk_r)

                for b in range(BLOC):
                    for jt in range(ST):
                        k_sb = ph1s.tile([128, DT, 128], bf16, tag="k_sb")
                        nc.sync.dma_start(k_sb[:, :, :], kT[b, jt])
                        ps_k = psum.tile([128, D], f32, tag="nd0")
                        for kt in range(DT):
                            nc.tensor.matmul(
                                ps_k[:, :],
                                k_sb[:, kt, :],
                                wk_sb[:, kt, :],
                                start=(kt == 0),
                                stop=(kt == DT - 1),
                            )
                        if use_kv:
                            kv_sb = ph1s.tile([128, DT, 128], bf16, tag="kv_sb")
                            nc.sync.dma_start(kv_sb[:, :, :], kTv[b, jt])
                        else:
                            kv_sb = k_sb
                        ps_v = psum.tile([128, D], f32, tag="nd2")
                        for kt in range(DT):
                            nc.tensor.matmul(
                                ps_v[:, :],
                                kv_sb[:, kt, :],
                                wvk_sb[:, kt, :],
                                start=(kt == 0),
                                stop=(kt == DT - 1),
                            )
                        nc.scalar.activation(E[b][:, jt, :], ps_k[:, :], ACT.Exp)
                        nc.vector.tensor_mul(
                            Ev[b][:, jt, :], E[b][:, jt, :], ps_v[:, :]
                        )

            # ---------------- phase 2: einsum + gating + output -------------
            with (
                tc.tile_pool(name="ph2", bufs=1) as ph2,
                tc.tile_pool(name="ph2pb", bufs=2) as ph2pb,
                tc.tile_pool(name="ph2s", bufs=3) as ph2s,
                tc.tile_pool(name="fin", bufs=2) as fin,
                tc.tile_pool(name="nds", bufs=2) as nds,
            ):
                wq_sb = ph2.tile([128, DT, D], bf16)
                nc.sync.dma_start(wq_sb[:, :, :], wq_r)
                wo_sb = ph2.tile([128, DT, D], bf16)
                nc.sync.dma_start(wo_sb[:, :, :], wo_r)

                # SBUF copies of num/den, double buffered across i-tiles
                def einsum_step(it):
                    pb_sb = ph2pb.tile([128, ST * 128], bf16, tag="pb_sb")
                    nc.sync.dma_start(
                        pb_sb[:, :], pbt[it].rearrange("p jt i -> p (jt i)")
                    )
                    ps_nd = [
                        psum.tile([128, D], f32, tag=f"nd{x}", name=f"nd{x}_{it}")
                        for x in range(2 * BLOC)
                    ]
                    for jt in range(ST):
                        lhs = pb_sb[:, jt * 128 : (jt + 1) * 128]
                        for b in range(BLOC):
                            nc.tensor.matmul(
                                ps_nd[2 * b][:, :],
                                lhs,
                                Ev[b][:, jt, :],
                                start=(jt == 0),
                                stop=(jt == ST - 1),
                            )
                            nc.tensor.matmul(
                                ps_nd[2 * b + 1][:, :],
                                lhs,
                                E[b][:, jt, :],
                                start=(jt == 0),
                                stop=(jt == ST - 1),
                            )
                    # evacuate to SBUF so the banks free up for the next i-tile
                    nd_sb = []
                    for x in range(2 * BLOC):
                        t = nds.tile([128, D], f32, tag=f"nds{x}", name=f"nds{x}_{it}")
                        nc.vector.tensor_copy(t[:, :], ps_nd[x][:, :])
                        nd_sb.append(t)
                    return nd_sb

                def finalize_step(it, nd_sb):
                    isl = slice(it * 128, (it + 1) * 128)
                    for b in range(BLOC):
                        q_sb = ph2s.tile([128, DT, 128], bf16, tag="q_sb")
                        nc.sync.dma_start(q_sb[:, :, :], qT[b, it])
                        ps_q = psum.tile([128, D], f32, tag="ps_q")
                        for kt in range(DT):
                            nc.tensor.matmul(
                                ps_q[:, :],
                                q_sb[:, kt, :],
                                wq_sb[:, kt, :],
                                start=(kt == 0),
                                stop=(kt == DT - 1),
                            )
                        sig = fin.tile([128, D], f32, tag="sig")
                        nc.scalar.activation(sig[:, :], ps_q[:, :], ACT.Sigmoid)
                        rec = fin.tile([128, D], f32, tag="rec")
                        nc.vector.reciprocal_approx_fast(
                            rec[:, :], nd_sb[2 * b + 1][:, :]
                        )
                        w = fin.tile([128, D], f32, tag="w")
                        nc.vector.tensor_mul(w[:, :], nd_sb[2 * b][:, :], rec[:, :])
                        y = fin.tile([128, D], bf16, tag="y")
                        nc.vector.tensor_mul(y[:, :], w[:, :], sig[:, :])
                        # transpose y -> yT [dk, s] (4 blocks into one PSUM bank)
                        ps_t = psum.tile([128, D], bf16, tag="ps_t")
                        for kt in range(DT):
                            nc.tensor.transpose(
                                ps_t[:, kt * 128 : (kt + 1) * 128],
                                y[:, kt * 128 : (kt + 1) * 128],
                                ident,
                            )
                        yT = fin.tile([128, DT, 128], bf16, tag="yT")
                        nc.vector.tensor_copy(
                            yT[:, :, :],
                            ps_t[:, :].rearrange("p (kt s) -> p kt s", kt=DT),
                        )
                        ps_o = psum.tile([128, D], f32, tag="ps_o")
                        for kt in range(DT):
                            nc.tensor.matmul(
                                ps_o[:, :],
                                yT[:, kt, :],
                                wo_sb[:, kt, :],
                                start=(kt == 0),
                                stop=(kt == DT - 1),
                            )
                        o_sb = fin.tile([128, D], f32, tag="o_sb")
                        nc.scalar.copy(o_sb[:, :], ps_o[:, :])
                        nc.sync.dma_start(out[b, isl, :], o_sb[:, :])

                # software pipeline: finalize(it-1) emitted after einsum(it)
                prev = None
                for it in range(NI):
                    nd_sb = einsum_step(it)
                    if prev is not None:
                        finalize_step(it - 1, prev)
                    prev = nd_sb
                finalize_step(NI - 1, prev)

    nc.compile()
    return nc


def _tile_act(xT):
    """[D, S] -> [st, p, kt, 128] host tiling (contiguous per (st))."""
    # xT[kt*128+p, st*128+sl] -> Z[st, p, kt, sl]
    z = xT.reshape(DT, 128, ST, 128)
    return np.ascontiguousarray(z.transpose(2, 1, 0, 3))


def _prep(query, key, Wq, bq, Wk, bk, Wv, bv, pos_bias, Wo, bo):
    """Host-side preprocessing: transposes, tiling, bias absorption, bf16."""
    import ml_dtypes

    bf16 = ml_dtypes.bfloat16

    query = np.asarray(query, dtype=np.float32)
    key = np.asarray(key, dtype=np.float32)
    Wq = np.asarray(Wq, dtype=np.float32)
    Wk = np.asarray(Wk, dtype=np.float32)
    Wv = np.asarray(Wv, dtype=np.float32)
    Wo = np.asarray(Wo, dtype=np.float32)
    bq = np.asarray(bq, dtype=np.float32)
    bk = np.asarray(bk, dtype=np.float32)
    bv = np.asarray(bv, dtype=np.float32)
    bo = np.asarray(bo, dtype=np.float32)

    Wvk = Wv @ Wk

    if np.any(bq):
        query = query + np.linalg.solve(Wq, bq).astype(np.float32)
    if np.any(bk):
        key_k = key + np.linalg.solve(Wk, bk).astype(np.float32)
    else:
        key_k = key
    use_kv = bool(np.any(bv)) or bool(np.any(bk))
    if use_kv:
        bv_eff = Wv @ bk + bv
        key_v = key + np.linalg.solve(Wvk, bv_eff).astype(np.float32)
    else:
        key_v = None

    # [S, B, D] -> per-batch [D, S] -> tiled [B, ST, 128, DT, 128] bf16
    qTb = query.transpose(1, 2, 0).astype(bf16)
    kTb = key_k.transpose(1, 2, 0).astype(bf16)
    qT = np.stack([_tile_act(qTb[b]) for b in range(B)])
    kT = np.stack([_tile_act(kTb[b]) for b in range(B)])
    if use_kv:
        kvb = key_v.transpose(1, 2, 0).astype(bf16)
        kTv = np.stack([_tile_act(kvb[b]) for b in range(B)])
    else:
        kTv = None

    # exp(pos_bias).T tiled: [it, p, jt, 128] = expPbT[jt*128+p, it*128+i]
    expPbT = np.exp(np.asarray(pos_bias, dtype=np.float32)).T.astype(bf16)
    pbt = np.ascontiguousarray(
        expPbT.reshape(ST, 128, NI, 128).transpose(2, 1, 0, 3)
    )

    wk = np.ascontiguousarray(Wk.T).astype(bf16)
    wvk = np.ascontiguousarray(Wvk.T).astype(bf16)
    wq = np.ascontiguousarray(Wq.T).astype(bf16)
    wo = np.ascontiguousarray(Wo.T).astype(bf16)
    return qT, kT, kTv, pbt, wk, wvk, wq, wo, bo, use_kv


def kernel(query, key, Wq, bq, Wk, bk, Wv, bv, pos_bias, Wo, bo):
    from concourse.bass_utils import run_bass_kernel_spmd

    qT, kT, kTv, pbt, wk, wvk, wq, wo, bo, use_kv = _prep(
        query, key, Wq, bq, Wk, bk, Wv, bv, pos_bias, Wo, bo
    )

    if ("nc", use_kv) not in _cache:
        _cache[("nc", use_kv)] = _build(use_kv)
    nc = _cache[("nc", use_kv)]

    in_maps = []
    for c in range(NCORES):
        bsl = slice(c * BLOC, (c + 1) * BLOC)
        m = {
            "qT": qT[bsl],
            "kT": kT[bsl],
            "pbt": pbt,
            "wk": wk,
            "wvk": wvk,
            "wq": wq,
            "wo": wo,
        }
        if use_kv:
            m["kTv"] = kTv[bsl]
        in_maps.append(m)

    res = run_bass_kernel_spmd(nc, in_maps, core_ids=list(range(NCORES)))
    out = np.concatenate([res.results[c]["out"] for c in range(NCORES)], axis=0)
    if np.any(bo):
        out = out + bo
    return out
